# revision 19
# baseline (speedup 1.0000x reference)
"""KoLeo loss kernel for 8 Trainium2 NeuronCores.

Reference semantics:
    v = latents.squeeze()            # [N, D] f32, N=16384, D=64
    dp = v @ v.T ; dp[i,i] = -1      # NxN scores, diagonal excluded
    idx = argmax(dp, axis=1)         # nearest neighbor by dot product
    dist = ||v - v[idx] + 1e-6||_2
    out = mean(relu(-log(dist * N)))

Sharding: rows are block-sharded 2048/core.  Each core gets a copy of v
whose rows are ROTATED by -core*2048, so the self-match diagonal of its
local [2048, 16384] score block always lands at pair-column (row//2) --
the SPMD program is identical on all cores.

Pairwise-max trick: max(a, b) = (a + b + |a-b|) / 2.  The host ships
paired sums w = v[0::2]+v[1::2] and diffs u = v[0::2]-v[1::2], both
PRE-SCALED by 2^17; the PE computes dp-sums s = rows @ w.T and dp-diffs
d = rows @ u.T.  ScalarE takes |d| out of PSUM (its only bulk job), and
a fused custom VectorE op consumes (s from PSUM, |d| from SBUF) at one
output/cycle -- i.e. TWO dp elements per DVE cycle:

    pack = round_16384(s + |d|) + global_pair_idx;  accum = max

The fp32 magic-constant rounding makes the pack exact; the scan carries
the GLOBAL pair index (wave base via the scan init), and the accumulator
is chained across waves through its seed operand, so a row-tile's final
bm value directly encodes (quantized pair-max, winning pair index) over
all 8192 pairs -- no per-chunk argmax recovery pass is needed.

The self-pair is excluded by accumulating -2^30 into the SUM stream at
the self position (one extra matmul per row-tile, wave 0 only); the
partner (the excluded pair's other member) is reinstated as an extra
candidate in the tail from host-precomputed partner dot/norm values.

Tail: one indirect gather per two row-tiles fetches the winning pair's
two member vectors plus their precomputed (norm - 2*eps*sum) values;
exact dots pick the member, a host-precomputed partner candidate is
merged, and dist^2 = hr + g - 2*dot, then relu(-0.5*ln - ln N).
Host: mean of the 8x2048 per-row values.
"""

import math

import ml_dtypes
import numpy as np

N = 16384
D = 64
NCORES = 8
ROWS = N // NCORES  # 2048 rows per core
P = 128  # partitions
NT = ROWS // P  # 16 row-tiles per core
NPAIR = N // 2  # 8192 pair columns
WCH = 1024  # pair columns per scan wave (2 PSUM banks)
NW = NPAIR // WCH  # 8 waves per row-tile
VREC = 160  # gather record width (f32): 640B, 64B-aligned
MM_N = 512  # matmul free dim (1 PSUM bank)
EPS = 1.0e-6

SCALE = 131072.0  # 2^17 pre-scale baked into wt/ut on the host
BIG = 2.0**30  # diagonal suppression on the (scaled) SUM stream
PACK_MAGIC = 3.0 * 2.0**22 * 16384.0  # rounds to multiples of 16384
UNPACK_MAGIC = 12582912.0  # 3 * 2^22: rounds to integers
NEGINF = -3.0e38

_OP_NAME = "KOLEO_PACK_GIDX"
_built = {}


def _register_pack_op():
    """Register the fused pair-max/global-argmax custom DVE op (idempotent).

    body  = round_16384(Src0 + Src1) + scan(+1, init=C1)
    accum = max, seeded from C0 (chains across waves via bm)
    Src0 = 2^17-scaled dp-sums (PSUM), Src1 = 2^17-scaled |dp-diffs| (SBUF).
    C1 = wave_base - 1 so the scan value is the GLOBAL pair index.
    C2 (imm2) = PACK_MAGIC.
    """
    from concourse import dve_ops
    from concourse.dve_spec import (
        AluOp, C0, C1, C2, One, Spec, Src0, Src1, lower, scan,
    )
    from concourse.dve_uop import DveOpSpec

    if _OP_NAME in dve_ops._SUB_OPCODE_FOR_NAME:
        return next(op for op in dve_ops.OPS if op.name == _OP_NAME)

    def _reference(in0, in1, s0, s1, imm2):
        p = in0.shape[0]
        s = in0.astype(np.float32).reshape(p, -1)
        a = in1.astype(np.float32).reshape(p, s.shape[1])
        z = (s + a) + np.float32(imm2) - np.float32(imm2)
        col = (np.float32(s1) + 1.0 + np.arange(s.shape[1], dtype=np.float32))[
            None, :
        ]
        body = (z + col).astype(np.float32)
        seed = np.asarray(s0, dtype=np.float32)
        seed = (
            seed.reshape(p, -1)[:, :1]
            if seed.ndim
            else np.full((p, 1), seed, dtype=np.float32)
        )
        acc = np.maximum(body.max(axis=-1, keepdims=True), seed)
        return body, acc

    body = ((Src0 + Src1) + C2 - C2) + scan(AluOp.ADD, One, init=C1)
    spec = Spec(
        body=body, accum=AluOp.MAX, accum_init=C0, reference=_reference
    )

    row = max(dve_ops._SUB_OPCODE_FOR_NAME.values()) + 1
    shas = {}
    for ver in ("v3", "v4"):
        uops = lower(spec, ver=ver)
        shas[ver] = DveOpSpec(
            name=_OP_NAME, opcode=row, uops=uops, rd1_en=True
        ).sha(ver)

    op = dve_ops.DveOp(_OP_NAME, spec, subdim=False, uops_sha=shas)
    dve_ops.OPS.append(op)
    dve_ops._SUB_OPCODE_FOR_NAME[_OP_NAME] = row
    dve_ops.CUSTOM_DVE_SPECS[_OP_NAME] = spec
    return op


def _build_nc():
    """Build + compile the per-core Bass program (same NEFF on all cores)."""
    if "nc" in _built:
        return _built["nc"]

    import concourse.bass as bass
    import concourse.mybir as mybir
    import concourse.tile as tile
    from concourse import bacc

    pack_op = _register_pack_op()

    f32 = mybir.dt.float32
    bf16 = mybir.dt.bfloat16
    i32 = mybir.dt.int32
    Alu = mybir.AluOpType
    Act = mybir.ActivationFunctionType

    nc = bacc.Bacc(None, target_bir_lowering=False)

    # wt/ut/vrows_t are duplicated into partitions 64-127 for row-packing
    wt_d = nc.declare_dram_parameter("wt", [P, NPAIR], bf16, isOutput=False)
    ut_d = nc.declare_dram_parameter("ut", [P, NPAIR], bf16, isOutput=False)
    vrows_t = nc.declare_dram_parameter("vrows_t", [P, ROWS], bf16, isOutput=False)
    # rows duplicated along D so one op handles both gathered pair members
    vr2_d = nc.declare_dram_parameter("vr2", [P, NT, 2 * D], f32, isOutput=False)
    # pair record: [v[2p] | v[2p+1] | g_a | g_b | pad pad] (132 f32 = 528B)
    vpair_d = nc.declare_dram_parameter("vpair", [NPAIR, VREC], f32, isOutput=False)
    hr_d = nc.declare_dram_parameter("hr", [P, NT], f32, isOutput=False)
    pd_d = nc.declare_dram_parameter("pd", [P, NT], f32, isOutput=False)
    pg_d = nc.declare_dram_parameter("pg", [P, NT], f32, isOutput=False)
    out_d = nc.declare_dram_parameter("out", [P, NT], f32, isOutput=True)
    if DEBUG:
        dbg_bm = nc.declare_dram_parameter("dbg_bm", [P, NT], f32, isOutput=True)
        dbg_pf = nc.declare_dram_parameter("dbg_pf", [P, NT], f32, isOutput=True)
        dbg_dq = nc.declare_dram_parameter("dbg_dq", [P, NT, 2], f32, isOutput=True)
        dbg_d2 = nc.declare_dram_parameter("dbg_d2", [P, NT], f32, isOutput=True)
        dbg_g = nc.declare_dram_parameter("dbg_g", [P, NT, 4], f32, isOutput=True)

    neg_eye = nc.inline_tensor(
        (np.eye(P, dtype=np.float32) * -BIG).astype(ml_dtypes.bfloat16), "neg_eye"
    )
    half_np = np.zeros((P, P // 2), dtype=np.float32)
    half_np[np.arange(P), np.arange(P) // 2] = 1.0
    half_eye = nc.inline_tensor(half_np.astype(ml_dtypes.bfloat16), "half_eye")
    neginf_np = np.full((P, NT), NEGINF, dtype=np.float32)
    neginf_c = nc.inline_tensor(neginf_np, "neginf")
    mlnn_np = np.full((P, 1), -math.log(float(N)), dtype=np.float32)
    mlnn_c = nc.inline_tensor(mlnn_np, "mlnn")

    with tile.TileContext(nc) as tc:
        with (
            tc.tile_pool(name="consts", bufs=1) as consts,
            tc.tile_pool(name="psum", bufs=1, space="PSUM") as psum_pool,
            tc.tile_pool(name="work", bufs=1) as work,
            tc.tile_pool(name="small", bufs=1) as small,
        ):
            # ---- startup DMAs: strict consumption order, spread over all
            # five engine queues (PE/DVE queues are idle at kernel start)
            negI_sb = consts.tile([P, P], bf16)
            nc.scalar.dma_start(negI_sb[:], neg_eye[:])
            halfI_sb = consts.tile([P, P // 2], bf16)
            nc.scalar.dma_start(halfI_sb[:], half_eye[:])
            neginf_sb = consts.tile([P, NT], f32)
            nc.scalar.dma_start(neginf_sb[:], neginf_c[:])
            mlnn_sb = consts.tile([P, 1], f32)
            nc.scalar.dma_start(mlnn_sb[:], mlnn_c[:])
            vrt_sb = consts.tile([P, ROWS], bf16)
            nc.sync.dma_start(vrt_sb[:, 0:256], vrows_t[:, 0:256])
            wt_sb = consts.tile([P, NPAIR], bf16)
            ut_sb = consts.tile([P, NPAIR], bf16)
            w0 = slice(0, WCH)
            w1 = slice(WCH, 2 * WCH)
            nc.gpsimd.dma_start(wt_sb[:, w0], wt_d[:, w0])
            nc.scalar.dma_start(ut_sb[:, w0], ut_d[:, w0])
            nc.sync.dma_start(vrt_sb[:, 256:ROWS], vrows_t[:, 256:ROWS])
            nc.gpsimd.dma_start(wt_sb[:, w1], wt_d[:, w1])
            nc.scalar.dma_start(ut_sb[:, w1], ut_d[:, w1])
            engs = [nc.sync, nc.gpsimd, nc.scalar]
            q = 0
            for w in range(2, NW):
                sl = slice(w * WCH, (w + 1) * WCH)
                engs[q % 3].dma_start(wt_sb[:, sl], wt_d[:, sl])
                q += 1
                engs[q % 3].dma_start(ut_sb[:, sl], ut_d[:, sl])
                q += 1
            vr2_sb = consts.tile([P, NT, 2 * D], f32)
            nc.gpsimd.dma_start(vr2_sb[:], vr2_d[:])
            hr_sb = consts.tile([P, NT], f32)
            nc.sync.dma_start(hr_sb[:], hr_d[:])
            pd_sb = consts.tile([P, NT], f32)
            nc.scalar.dma_start(pd_sb[:], pd_d[:])
            pg_sb = consts.tile([P, NT], f32)
            nc.gpsimd.dma_start(pg_sb[:], pg_d[:])

            bm = small.tile([P, NT], f32)  # packed per-tile running maxima
            u1 = small.tile([P, NT], f32)
            u2 = small.tile([P, NT], f32)
            pff = small.tile([P, NT], f32)  # winning pair index (float)
            pfi = small.tile([P, NT], i32)  # winning pair index (int)
            gat = small.tile([P, NT, VREC], f32)  # gathered pair records
            prj = small.tile([P, NT, 2, D], f32)  # member products
            dq = small.tile([P, NT, 2], f32)  # member dots
            sel = small.tile([P, NT], f32)  # 1.0 if member b has larger dot
            ddf = small.tile([P, NT], f32)
            dm = small.tile([P, NT], f32)
            dotw = small.tile([P, NT], f32)
            gdf = small.tile([P, NT], f32)
            gm = small.tile([P, NT], f32)
            gw = small.tile([P, NT], f32)
            c2 = small.tile([P, NT], f32)
            pdd = small.tile([P, NT], f32)
            pdm = small.tile([P, NT], f32)
            dotf = small.tile([P, NT], f32)
            pgd = small.tile([P, NT], f32)
            pgm = small.tile([P, NT], f32)
            gf = small.tile([P, NT], f32)
            s2a = small.tile([P, NT], f32)
            dist2 = small.tile([P, NT], f32)

            def wave_pair(s, w):
                tA, tB = 2 * s, 2 * s + 1
                lhsA = vrt_sb[0:64, tA * P : (tA + 1) * P]
                lhsB = vrt_sb[64:128, tB * P : (tB + 1) * P]
                if True:
                    # A uses PE rows 0-63, B rows 64-127: adjacent MMs
                    # alternate quadrants so pairs run concurrently.  The 4
                    # single-buffered tag-tiles fill all 8 PSUM banks; A and
                    # B act as each other's double buffer in the pipeline.
                    psSA = psum_pool.tile([P, WCH], f32, tag="psSA", bufs=1)
                    psSB = psum_pool.tile([P, WCH], f32, tag="psSB", bufs=1)
                    psDA = psum_pool.tile([P, WCH], f32, tag="psDA", bufs=1)
                    psDB = psum_pool.tile([P, WCH], f32, tag="psDB", bufs=1)
                    for h in range(WCH // MM_N):
                        csl = slice(w * WCH + h * MM_N, w * WCH + (h + 1) * MM_N)
                        osl = slice(h * MM_N, (h + 1) * MM_N)
                        nc.tensor.matmul(
                            psDA[:, osl], lhsA, ut_sb[0:64, csl],
                            start=True, stop=True,
                        )
                        nc.tensor.matmul(
                            psDB[:, osl], lhsB, ut_sb[64:128, csl],
                            start=True, stop=True,
                        )
                    absA = work.tile([P, WCH], f32, tag="absD", bufs=4)
                    nc.scalar.activation(absA[:], psDA[:], Act.Abs)
                    for h in range(WCH // MM_N):
                        csl = slice(w * WCH + h * MM_N, w * WCH + (h + 1) * MM_N)
                        osl = slice(h * MM_N, (h + 1) * MM_N)
                        # the self-pair block (cols [t*64, t*64+64)) is
                        # always inside wave 0; keep that MM's group open
                        dhA = w == 0 and h == (tA * 64) // MM_N
                        dhB = w == 0 and h == (tB * 64) // MM_N
                        nc.tensor.matmul(
                            psSA[:, osl], lhsA, wt_sb[0:64, csl],
                            start=True, stop=not dhA,
                        )
                        nc.tensor.matmul(
                            psSB[:, osl], lhsB, wt_sb[64:128, csl],
                            start=True, stop=not dhB,
                        )
                    if w == 0:
                        offA, offB = tA * 64, tB * 64
                        nc.tensor.matmul(
                            psSA[:, offA : offA + 64], negI_sb[:], halfI_sb[:],
                            start=False, stop=True,
                        )
                        nc.tensor.matmul(
                            psSB[:, offB : offB + 64], negI_sb[:], halfI_sb[:],
                            start=False, stop=True,
                        )
                    absB = work.tile([P, WCH], f32, tag="absD", bufs=4)
                    nc.scalar.activation(absB[:], psDB[:], Act.Abs)
                    for t, psS, absD in ((tA, psSA, absA), (tB, psSB, absB)):
                        junk = work.tile([P, WCH], f32, tag="junk", bufs=2)
                        seed = (
                            neginf_sb[:, t : t + 1]
                            if w == 0
                            else bm[:, t : t + 1]
                        )
                        nc.vector._custom_dve(
                            pack_op, out=junk[:], in0=psS[:], in1=absD[:],
                            s0=seed, s1=float(w * WCH - 1), imm2=PACK_MAGIC,
                            accum_out=bm[:, t : t + 1],
                        )

            # waves 0-1 wave-major across all tiles: compute starts as soon
            # as the first 1MB of wt/ut lands, covering the remaining DMAs
            for w in range(2):
                for s in range(NT // 2):
                    wave_pair(s, w)
            for s in range(NT // 2):
                tA, tB = 2 * s, 2 * s + 1
                for w in range(2, NW):
                    wave_pair(s, w)

                # ---- winning pair index for tiles (tA, tB) + gather
                tsl = slice(tA, tB + 1)
                nc.scalar.activation(
                    u1[:, tsl], bm[:, tsl], Act.Copy,
                    bias=UNPACK_MAGIC, scale=1.0 / 16384.0,
                )
                nc.scalar.activation(
                    u2[:, tsl], u1[:, tsl], Act.Copy,
                    bias=16384.0 * UNPACK_MAGIC, scale=-16384.0,
                )
                nc.vector.tensor_tensor(
                    out=pff[:, tsl], in0=bm[:, tsl], in1=u2[:, tsl], op=Alu.add
                )
                nc.vector.tensor_copy(pfi[:, tsl], pff[:, tsl])
                for t in (tA, tB):
                    nc.gpsimd.indirect_dma_start(
                        out=gat[:, t, :], out_offset=None, in_=vpair_d[:],
                        in_offset=bass.IndirectOffsetOnAxis(
                            ap=pfi[:, t : t + 1], axis=0
                        ),
                    )
                if s % 2 == 1:
                    # ---- exact dots + candidate combine for the last 4 tiles
                    g4 = slice(tA - 2, tB + 1)
                    nc.vector.tensor_tensor(
                        out=prj[:, g4, :, :],
                        in0=vr2_sb[:, g4, :],
                        in1=gat[:, g4, 0 : 2 * D],
                        op=Alu.mult,
                    )
                    nc.vector.tensor_reduce(
                        dq[:, g4, :], prj[:, g4, :, :],
                        axis=mybir.AxisListType.X, op=Alu.add,
                    )
                    # member with larger dot
                    nc.vector.tensor_tensor(
                        out=sel[:, g4], in0=dq[:, g4, 1], in1=dq[:, g4, 0],
                        op=Alu.is_gt,
                    )
                    nc.vector.tensor_tensor(
                        out=ddf[:, g4], in0=dq[:, g4, 1], in1=dq[:, g4, 0],
                        op=Alu.subtract,
                    )
                    nc.vector.tensor_tensor(
                        out=dm[:, g4], in0=ddf[:, g4], in1=sel[:, g4], op=Alu.mult
                    )
                    nc.vector.tensor_tensor(
                        out=dotw[:, g4], in0=dq[:, g4, 0], in1=dm[:, g4], op=Alu.add
                    )
                    nc.vector.tensor_tensor(
                        out=gdf[:, g4], in0=gat[:, g4, 2 * D + 1],
                        in1=gat[:, g4, 2 * D], op=Alu.subtract,
                    )
                    nc.vector.tensor_tensor(
                        out=gm[:, g4], in0=gdf[:, g4], in1=sel[:, g4], op=Alu.mult
                    )
                    nc.vector.tensor_tensor(
                        out=gw[:, g4], in0=gat[:, g4, 2 * D], in1=gm[:, g4],
                        op=Alu.add,
                    )
                    # partner candidate wins if its (host-exact) dot is larger
                    nc.vector.tensor_tensor(
                        out=c2[:, g4], in0=pd_sb[:, g4], in1=dotw[:, g4],
                        op=Alu.is_gt,
                    )
                    nc.vector.tensor_tensor(
                        out=pdd[:, g4], in0=pd_sb[:, g4], in1=dotw[:, g4],
                        op=Alu.subtract,
                    )
                    nc.vector.tensor_tensor(
                        out=pdm[:, g4], in0=pdd[:, g4], in1=c2[:, g4], op=Alu.mult
                    )
                    nc.vector.tensor_tensor(
                        out=dotf[:, g4], in0=dotw[:, g4], in1=pdm[:, g4], op=Alu.add
                    )
                    nc.vector.tensor_tensor(
                        out=pgd[:, g4], in0=pg_sb[:, g4], in1=gw[:, g4],
                        op=Alu.subtract,
                    )
                    nc.vector.tensor_tensor(
                        out=pgm[:, g4], in0=pgd[:, g4], in1=c2[:, g4], op=Alu.mult
                    )
                    nc.vector.tensor_tensor(
                        out=gf[:, g4], in0=gw[:, g4], in1=pgm[:, g4], op=Alu.add
                    )
                    # dist^2 = hr + g - 2*dot
                    nc.vector.scalar_tensor_tensor(
                        out=s2a[:, g4], in0=dotf[:, g4], scalar=-2.0,
                        in1=gf[:, g4], op0=Alu.mult, op1=Alu.add,
                    )
                    nc.vector.tensor_tensor(
                        out=dist2[:, g4], in0=s2a[:, g4], in1=hr_sb[:, g4],
                        op=Alu.add,
                    )
            if DEBUG:
                nc.sync.dma_start(dbg_bm[:], bm[:])
                nc.sync.dma_start(dbg_pf[:], pff[:])
                nc.sync.dma_start(dbg_dq[:], dq[:])
                nc.sync.dma_start(dbg_d2[:], dist2[:])
                nc.sync.dma_start(dbg_g[:, :, 0], gat[:, :, 128])
                nc.sync.dma_start(dbg_g[:, :, 1], gat[:, :, 129])
                nc.sync.dma_start(dbg_g[:, :, 2], gat[:, :, 0])
                nc.sync.dma_start(dbg_g[:, :, 3], gat[:, :, D])
            lns = small.tile([P, NT], f32)
            nc.scalar.activation(lns[:], dist2[:], Act.Ln)
            kol = small.tile([P, NT], f32)
            # koleo = relu(-0.5*ln(dist^2) - ln(N))
            nc.scalar.activation(
                kol[:], lns[:], Act.Relu, bias=mlnn_sb[:, 0:1], scale=-0.5
            )
            nc.sync.dma_start(out_d[:], kol[:])

    nc.compile()
    _built["nc"] = nc
    return nc


def _prep_in_maps(v: np.ndarray) -> list[dict]:
    bf = ml_dtypes.bfloat16
    n2_all = None
    in_maps = []
    jj = np.arange(ROWS)
    for c in range(NCORES):
        vr = np.roll(v, -c * ROWS, axis=0)
        w = (vr[0::2] + vr[1::2]) * np.float32(SCALE)  # [NPAIR, D]
        u = (vr[0::2] - vr[1::2]) * np.float32(SCALE)
        rows = vr[:ROWS]
        wt = np.ascontiguousarray(w.T).astype(bf)
        ut = np.ascontiguousarray(u.T).astype(bf)
        rt = np.ascontiguousarray(rows.T).astype(bf)
        rsb = rows.reshape(NT, P, D).transpose(1, 0, 2)

        n2 = np.sum(vr.astype(np.float64) * vr, axis=1).astype(np.float32)
        Sv = np.sum(vr.astype(np.float64), axis=1).astype(np.float32)
        gvec = n2 - np.float32(2.0 * EPS) * Sv  # per-row: ||x||^2 - 2 eps sum(x)

        vpair = np.zeros((NPAIR, VREC), dtype=np.float32)
        vpair[:, 0:D] = vr[0::2]
        vpair[:, D : 2 * D] = vr[1::2]
        vpair[:, 2 * D] = gvec[0::2]
        vpair[:, 2 * D + 1] = gvec[1::2]

        hr = (
            n2[:ROWS]
            + np.float32(2.0 * EPS) * Sv[:ROWS]
            + np.float32(D * EPS * EPS)
        )
        pidx = jj ^ 1
        pdot = np.sum(
            rows.astype(np.float64) * vr[pidx], axis=1
        ).astype(np.float32)
        pgv = gvec[pidx]

        in_maps.append(
            {
                "wt": np.concatenate([wt, wt], axis=0),
                "ut": np.concatenate([ut, ut], axis=0),
                "vrows_t": np.concatenate([rt, rt], axis=0),
                "vr2": np.ascontiguousarray(np.concatenate([rsb, rsb], axis=2)),
                "vpair": vpair,
                "hr": np.ascontiguousarray(hr.reshape(NT, P).T),
                "pd": np.ascontiguousarray(pdot.reshape(NT, P).T),
                "pg": np.ascontiguousarray(pgv.reshape(NT, P).T),
            }
        )
    return in_maps


# test.py can flip these to profile the run
TRACE = False
DEBUG = False
DEBUG_NO_CLAMP = False
LAST_RESULT = {}


def kernel(latents: np.ndarray) -> np.ndarray:
    from concourse.bass_utils import run_bass_kernel_spmd

    v = np.asarray(latents, dtype=np.float32).reshape(N, D)
    nc = _build_nc()
    in_maps = _prep_in_maps(v)

    kwargs = {}
    if TRACE:
        kwargs = dict(trace=True, stitch_traces=False)
    res = run_bass_kernel_spmd(nc, in_maps, core_ids=list(range(NCORES)), **kwargs)
    LAST_RESULT["res"] = res

    vals = np.concatenate([r["out"].reshape(-1) for r in res.results])
    return np.array(np.mean(vals), dtype=np.float32)


# revision 20
# speedup vs baseline: 1.1992x; 1.1992x over previous
"""KoLeo loss kernel for 8 Trainium2 NeuronCores.

Reference semantics:
    v = latents.squeeze()            # [N, D] f32, N=16384, D=64
    dp = v @ v.T ; dp[i,i] = -1      # NxN scores, diagonal excluded
    idx = argmax(dp, axis=1)         # nearest neighbor by dot product
    dist = ||v - v[idx] + 1e-6||_2
    out = mean(relu(-log(dist * N)))

Sharding: rows are block-sharded 2048/core.  Each core gets a copy of v
whose rows are ROTATED by -core*2048, so the self-match diagonal of its
local [2048, 16384] score block always lands at pair-column (row//2) --
the SPMD program is identical on all cores.

Pairwise-max trick: max(a, b) = (a + b + |a-b|) / 2.  The host ships
paired sums w = v[0::2]+v[1::2] and diffs u = v[0::2]-v[1::2], both
PRE-SCALED by 2^17; the PE computes dp-sums s = rows @ w.T and dp-diffs
d = rows @ u.T.  ScalarE takes |d| out of PSUM (its only bulk job), and
a fused custom VectorE op consumes (s from PSUM, |d| from SBUF) at one
output/cycle -- i.e. TWO dp elements per DVE cycle:

    pack = round_16384(s + |d|) + global_pair_idx;  accum = max

The fp32 magic-constant rounding makes the pack exact; the scan carries
the GLOBAL pair index (wave base via the scan init), and the accumulator
is chained across waves through its seed operand, so a row-tile's final
bm value directly encodes (quantized pair-max, winning pair index) over
all 8192 pairs -- no per-chunk argmax recovery pass is needed.

The self-pair is excluded by accumulating -2^30 into the SUM stream at
the self position (one extra matmul per row-tile, wave 0 only); the
partner (the excluded pair's other member) is reinstated as an extra
candidate in the tail from host-precomputed partner dot/norm values.

Tail: one indirect gather per two row-tiles fetches the winning pair's
two member vectors plus their precomputed (norm - 2*eps*sum) values;
exact dots pick the member, a host-precomputed partner candidate is
merged, and dist^2 = hr + g - 2*dot, then relu(-0.5*ln - ln N).
Host: mean of the 8x2048 per-row values.
"""

import math

import ml_dtypes
import numpy as np

N = 16384
D = 64
NCORES = 8
ROWS = N // NCORES  # 2048 rows per core
P = 128  # partitions
NT = ROWS // P  # 16 row-tiles per core
NPAIR = N // 2  # 8192 pair columns
WCH = 1024  # pair columns per scan wave (2 PSUM banks)
NW = NPAIR // WCH  # 8 waves per row-tile
VREC = 160  # gather record width (f32): 640B, 64B-aligned
MM_N = 512  # matmul free dim (1 PSUM bank)
EPS = 1.0e-6

SCALE = 131072.0  # 2^17 pre-scale baked into wt/ut on the host
BIG = 2.0**30  # diagonal suppression on the (scaled) SUM stream
PACK_MAGIC = 3.0 * 2.0**22 * 16384.0  # rounds to multiples of 16384
UNPACK_MAGIC = 12582912.0  # 3 * 2^22: rounds to integers
NEGINF = -3.0e38

_OP_NAME = "KOLEO_PACK_GIDX"
_built = {}


def _register_pack_op():
    """Register the fused pair-max/global-argmax custom DVE op (idempotent).

    body  = round_16384(Src0 + Src1) + scan(+1, init=C1)
    accum = max, seeded from C0 (chains across waves via bm)
    Src0 = 2^17-scaled dp-sums (PSUM), Src1 = 2^17-scaled |dp-diffs| (SBUF).
    C1 = wave_base - 1 so the scan value is the GLOBAL pair index.
    C2 (imm2) = PACK_MAGIC.
    """
    from concourse import dve_ops
    from concourse.dve_spec import (
        AluOp, C0, C1, C2, One, Spec, Src0, Src1, lower, scan,
    )
    from concourse.dve_uop import DveOpSpec

    if _OP_NAME in dve_ops._SUB_OPCODE_FOR_NAME:
        return next(op for op in dve_ops.OPS if op.name == _OP_NAME)

    def _reference(in0, in1, s0, s1, imm2):
        p = in0.shape[0]
        s = in0.astype(np.float32).reshape(p, -1)
        a = in1.astype(np.float32).reshape(p, s.shape[1])
        z = (s + a) + np.float32(imm2) - np.float32(imm2)
        col = (np.float32(s1) + 1.0 + np.arange(s.shape[1], dtype=np.float32))[
            None, :
        ]
        body = (z + col).astype(np.float32)
        seed = np.asarray(s0, dtype=np.float32)
        seed = (
            seed.reshape(p, -1)[:, :1]
            if seed.ndim
            else np.full((p, 1), seed, dtype=np.float32)
        )
        acc = np.maximum(body.max(axis=-1, keepdims=True), seed)
        return body, acc

    body = ((Src0 + Src1) + C2 - C2) + scan(AluOp.ADD, One, init=C1)
    spec = Spec(
        body=body, accum=AluOp.MAX, accum_init=C0, reference=_reference
    )

    row = max(dve_ops._SUB_OPCODE_FOR_NAME.values()) + 1
    shas = {}
    for ver in ("v3", "v4"):
        uops = lower(spec, ver=ver)
        shas[ver] = DveOpSpec(
            name=_OP_NAME, opcode=row, uops=uops, rd1_en=True
        ).sha(ver)

    op = dve_ops.DveOp(_OP_NAME, spec, subdim=False, uops_sha=shas)
    dve_ops.OPS.append(op)
    dve_ops._SUB_OPCODE_FOR_NAME[_OP_NAME] = row
    dve_ops.CUSTOM_DVE_SPECS[_OP_NAME] = spec
    return op


def _build_nc():
    """Build + compile the per-core Bass program (same NEFF on all cores)."""
    if "nc" in _built:
        return _built["nc"]

    import concourse.bass as bass
    import concourse.mybir as mybir
    import concourse.tile as tile
    from concourse import bacc

    pack_op = _register_pack_op()

    f32 = mybir.dt.float32
    bf16 = mybir.dt.bfloat16
    i32 = mybir.dt.int32
    Alu = mybir.AluOpType
    Act = mybir.ActivationFunctionType

    nc = bacc.Bacc(None, target_bir_lowering=False)

    # wt/ut/vrows_t are duplicated into partitions 64-127 for row-packing
    wt_d = nc.declare_dram_parameter("wt", [P, NPAIR], bf16, isOutput=False)
    ut_d = nc.declare_dram_parameter("ut", [P, NPAIR], bf16, isOutput=False)
    vrows_t = nc.declare_dram_parameter("vrows_t", [P, ROWS], bf16, isOutput=False)
    # rows duplicated along D so one op handles both gathered pair members
    vr2_d = nc.declare_dram_parameter("vr2", [P, NT, 2 * D], f32, isOutput=False)
    # pair record: [v[2p] | v[2p+1] | g_a | g_b | pad pad] (132 f32 = 528B)
    vpair_d = nc.declare_dram_parameter("vpair", [NPAIR, VREC], f32, isOutput=False)
    hr_d = nc.declare_dram_parameter("hr", [P, NT], f32, isOutput=False)
    pd_d = nc.declare_dram_parameter("pd", [P, NT], f32, isOutput=False)
    pg_d = nc.declare_dram_parameter("pg", [P, NT], f32, isOutput=False)
    out_d = nc.declare_dram_parameter("out", [P, NT], f32, isOutput=True)
    if DEBUG:
        dbg_bm = nc.declare_dram_parameter("dbg_bm", [P, NT], f32, isOutput=True)
        dbg_pf = nc.declare_dram_parameter("dbg_pf", [P, NT], f32, isOutput=True)
        dbg_dq = nc.declare_dram_parameter("dbg_dq", [P, NT, 2], f32, isOutput=True)
        dbg_d2 = nc.declare_dram_parameter("dbg_d2", [P, NT], f32, isOutput=True)
        dbg_g = nc.declare_dram_parameter("dbg_g", [P, NT, 4], f32, isOutput=True)

    neg_eye = nc.inline_tensor(
        (np.eye(P, dtype=np.float32) * -BIG).astype(ml_dtypes.bfloat16), "neg_eye"
    )
    half_np = np.zeros((P, P // 2), dtype=np.float32)
    half_np[np.arange(P), np.arange(P) // 2] = 1.0
    half_eye = nc.inline_tensor(half_np.astype(ml_dtypes.bfloat16), "half_eye")
    neginf_np = np.full((P, NT), NEGINF, dtype=np.float32)
    neginf_c = nc.inline_tensor(neginf_np, "neginf")
    mlnn_np = np.full((P, 1), -math.log(float(N)), dtype=np.float32)
    mlnn_c = nc.inline_tensor(mlnn_np, "mlnn")

    with tile.TileContext(nc) as tc:
        with (
            tc.tile_pool(name="consts", bufs=1) as consts,
            tc.tile_pool(name="psum", bufs=1, space="PSUM") as psum_pool,
            tc.tile_pool(name="work", bufs=1) as work,
            tc.tile_pool(name="small", bufs=1) as small,
        ):
            # ---- startup DMAs: strict consumption order, spread over all
            # five engine queues (PE/DVE queues are idle at kernel start)
            negI_sb = consts.tile([P, P], bf16)
            nc.scalar.dma_start(negI_sb[:], neg_eye[:])
            halfI_sb = consts.tile([P, P // 2], bf16)
            nc.scalar.dma_start(halfI_sb[:], half_eye[:])
            neginf_sb = consts.tile([P, NT], f32)
            nc.scalar.dma_start(neginf_sb[:], neginf_c[:])
            mlnn_sb = consts.tile([P, 1], f32)
            nc.scalar.dma_start(mlnn_sb[:], mlnn_c[:])
            vrt_sb = consts.tile([P, ROWS], bf16)
            nc.sync.dma_start(vrt_sb[:, 0:256], vrows_t[:, 0:256])
            wt_sb = consts.tile([P, NPAIR], bf16)
            ut_sb = consts.tile([P, NPAIR], bf16)
            w0 = slice(0, WCH)
            w1 = slice(WCH, 2 * WCH)
            nc.gpsimd.dma_start(wt_sb[:, w0], wt_d[:, w0])
            nc.scalar.dma_start(ut_sb[:, w0], ut_d[:, w0])
            nc.sync.dma_start(vrt_sb[:, 256:ROWS], vrows_t[:, 256:ROWS])
            nc.gpsimd.dma_start(wt_sb[:, w1], wt_d[:, w1])
            nc.scalar.dma_start(ut_sb[:, w1], ut_d[:, w1])
            engs = [nc.sync, nc.gpsimd, nc.scalar]
            q = 0
            for w in range(2, NW):
                sl = slice(w * WCH, (w + 1) * WCH)
                engs[q % 3].dma_start(wt_sb[:, sl], wt_d[:, sl])
                q += 1
                engs[q % 3].dma_start(ut_sb[:, sl], ut_d[:, sl])
                q += 1
            vr2_sb = consts.tile([P, NT, 2 * D], f32)
            nc.gpsimd.dma_start(vr2_sb[:], vr2_d[:])
            hr_sb = consts.tile([P, NT], f32)
            nc.sync.dma_start(hr_sb[:], hr_d[:])
            pd_sb = consts.tile([P, NT], f32)
            nc.scalar.dma_start(pd_sb[:], pd_d[:])
            pg_sb = consts.tile([P, NT], f32)
            nc.gpsimd.dma_start(pg_sb[:], pg_d[:])

            bm = small.tile([P, NT], f32)  # packed per-tile running maxima
            u1 = small.tile([P, NT], f32)
            u2 = small.tile([P, NT], f32)
            pff = small.tile([P, NT], f32)  # winning pair index (float)
            pfi = small.tile([P, NT], i32)  # winning pair index (int)
            gat = small.tile([P, NT, VREC], f32)  # gathered pair records
            prj = small.tile([P, NT, 2, D], f32)  # member products
            dq = small.tile([P, NT, 2], f32)  # member dots
            sel = small.tile([P, NT], f32)  # 1.0 if member b has larger dot
            ddf = small.tile([P, NT], f32)
            dm = small.tile([P, NT], f32)
            dotw = small.tile([P, NT], f32)
            gdf = small.tile([P, NT], f32)
            gm = small.tile([P, NT], f32)
            gw = small.tile([P, NT], f32)
            c2 = small.tile([P, NT], f32)
            pdd = small.tile([P, NT], f32)
            pdm = small.tile([P, NT], f32)
            dotf = small.tile([P, NT], f32)
            pgd = small.tile([P, NT], f32)
            pgm = small.tile([P, NT], f32)
            gf = small.tile([P, NT], f32)
            s2a = small.tile([P, NT], f32)
            dist2 = small.tile([P, NT], f32)

            def wave_pair(s, w):
                tA, tB = 2 * s, 2 * s + 1
                lhsA = vrt_sb[0:64, tA * P : (tA + 1) * P]
                lhsB = vrt_sb[64:128, tB * P : (tB + 1) * P]
                if True:
                    # A uses PE rows 0-63, B rows 64-127: adjacent MMs
                    # alternate quadrants so pairs run concurrently.  The 4
                    # single-buffered tag-tiles fill all 8 PSUM banks; A and
                    # B act as each other's double buffer in the pipeline.
                    psSA = psum_pool.tile([P, WCH], f32, tag="psSA", bufs=1)
                    psSB = psum_pool.tile([P, WCH], f32, tag="psSB", bufs=1)
                    psDA = psum_pool.tile([P, WCH], f32, tag="psDA", bufs=1)
                    psDB = psum_pool.tile([P, WCH], f32, tag="psDB", bufs=1)
                    for h in range(WCH // MM_N):
                        csl = slice(w * WCH + h * MM_N, w * WCH + (h + 1) * MM_N)
                        osl = slice(h * MM_N, (h + 1) * MM_N)
                        nc.tensor.matmul(
                            psDA[:, osl], lhsA, ut_sb[0:64, csl],
                            start=True, stop=True,
                        )
                        nc.tensor.matmul(
                            psDB[:, osl], lhsB, ut_sb[64:128, csl],
                            start=True, stop=True,
                        )
                    absA = work.tile([P, WCH], f32, tag="absD", bufs=4)
                    nc.scalar.activation(absA[:], psDA[:], Act.Abs)
                    for h in range(WCH // MM_N):
                        csl = slice(w * WCH + h * MM_N, w * WCH + (h + 1) * MM_N)
                        osl = slice(h * MM_N, (h + 1) * MM_N)
                        # the self-pair block (cols [t*64, t*64+64)) is
                        # always inside wave 0; keep that MM's group open
                        dhA = w == 0 and h == (tA * 64) // MM_N
                        dhB = w == 0 and h == (tB * 64) // MM_N
                        nc.tensor.matmul(
                            psSA[:, osl], lhsA, wt_sb[0:64, csl],
                            start=True, stop=not dhA,
                        )
                        nc.tensor.matmul(
                            psSB[:, osl], lhsB, wt_sb[64:128, csl],
                            start=True, stop=not dhB,
                        )
                    if w == 0:
                        offA, offB = tA * 64, tB * 64
                        nc.tensor.matmul(
                            psSA[:, offA : offA + 64], negI_sb[:], halfI_sb[:],
                            start=False, stop=True,
                        )
                        nc.tensor.matmul(
                            psSB[:, offB : offB + 64], negI_sb[:], halfI_sb[:],
                            start=False, stop=True,
                        )
                    absB = work.tile([P, WCH], f32, tag="absD", bufs=4)
                    nc.scalar.activation(absB[:], psDB[:], Act.Abs)
                    for t, psS, absD in ((tA, psSA, absA), (tB, psSB, absB)):
                        junk = work.tile([P, WCH], f32, tag="junk", bufs=2)
                        seed = (
                            neginf_sb[:, t : t + 1]
                            if w == 0
                            else bm[:, t : t + 1]
                        )
                        nc.vector._custom_dve(
                            pack_op, out=junk[:], in0=psS[:], in1=absD[:],
                            s0=seed, s1=float(w * WCH - 1), imm2=PACK_MAGIC,
                            accum_out=bm[:, t : t + 1],
                        )

            for s in range(NT // 2):
                tA, tB = 2 * s, 2 * s + 1
                for w in range(NW):
                    wave_pair(s, w)

                # ---- winning pair index for tiles (tA, tB) + gather
                tsl = slice(tA, tB + 1)
                nc.scalar.activation(
                    u1[:, tsl], bm[:, tsl], Act.Copy,
                    bias=UNPACK_MAGIC, scale=1.0 / 16384.0,
                )
                nc.scalar.activation(
                    u2[:, tsl], u1[:, tsl], Act.Copy,
                    bias=16384.0 * UNPACK_MAGIC, scale=-16384.0,
                )
                nc.vector.tensor_tensor(
                    out=pff[:, tsl], in0=bm[:, tsl], in1=u2[:, tsl], op=Alu.add
                )
                nc.vector.tensor_copy(pfi[:, tsl], pff[:, tsl])
                for t in (tA, tB):
                    nc.gpsimd.indirect_dma_start(
                        out=gat[:, t, :], out_offset=None, in_=vpair_d[:],
                        in_offset=bass.IndirectOffsetOnAxis(
                            ap=pfi[:, t : t + 1], axis=0
                        ),
                    )
                if s % 2 == 1:
                    # ---- exact dots + candidate combine for the last 4 tiles
                    g4 = slice(tA - 2, tB + 1)
                    nc.vector.tensor_tensor(
                        out=prj[:, g4, :, :],
                        in0=vr2_sb[:, g4, :],
                        in1=gat[:, g4, 0 : 2 * D],
                        op=Alu.mult,
                    )
                    nc.vector.tensor_reduce(
                        dq[:, g4, :], prj[:, g4, :, :],
                        axis=mybir.AxisListType.X, op=Alu.add,
                    )
                    # member with larger dot
                    nc.vector.tensor_tensor(
                        out=sel[:, g4], in0=dq[:, g4, 1], in1=dq[:, g4, 0],
                        op=Alu.is_gt,
                    )
                    nc.vector.tensor_tensor(
                        out=ddf[:, g4], in0=dq[:, g4, 1], in1=dq[:, g4, 0],
                        op=Alu.subtract,
                    )
                    nc.vector.tensor_tensor(
                        out=dm[:, g4], in0=ddf[:, g4], in1=sel[:, g4], op=Alu.mult
                    )
                    nc.vector.tensor_tensor(
                        out=dotw[:, g4], in0=dq[:, g4, 0], in1=dm[:, g4], op=Alu.add
                    )
                    nc.vector.tensor_tensor(
                        out=gdf[:, g4], in0=gat[:, g4, 2 * D + 1],
                        in1=gat[:, g4, 2 * D], op=Alu.subtract,
                    )
                    nc.vector.tensor_tensor(
                        out=gm[:, g4], in0=gdf[:, g4], in1=sel[:, g4], op=Alu.mult
                    )
                    nc.vector.tensor_tensor(
                        out=gw[:, g4], in0=gat[:, g4, 2 * D], in1=gm[:, g4],
                        op=Alu.add,
                    )
                    # partner candidate wins if its (host-exact) dot is larger
                    nc.vector.tensor_tensor(
                        out=c2[:, g4], in0=pd_sb[:, g4], in1=dotw[:, g4],
                        op=Alu.is_gt,
                    )
                    nc.vector.tensor_tensor(
                        out=pdd[:, g4], in0=pd_sb[:, g4], in1=dotw[:, g4],
                        op=Alu.subtract,
                    )
                    nc.vector.tensor_tensor(
                        out=pdm[:, g4], in0=pdd[:, g4], in1=c2[:, g4], op=Alu.mult
                    )
                    nc.vector.tensor_tensor(
                        out=dotf[:, g4], in0=dotw[:, g4], in1=pdm[:, g4], op=Alu.add
                    )
                    nc.vector.tensor_tensor(
                        out=pgd[:, g4], in0=pg_sb[:, g4], in1=gw[:, g4],
                        op=Alu.subtract,
                    )
                    nc.vector.tensor_tensor(
                        out=pgm[:, g4], in0=pgd[:, g4], in1=c2[:, g4], op=Alu.mult
                    )
                    nc.vector.tensor_tensor(
                        out=gf[:, g4], in0=gw[:, g4], in1=pgm[:, g4], op=Alu.add
                    )
                    # dist^2 = hr + g - 2*dot
                    nc.vector.scalar_tensor_tensor(
                        out=s2a[:, g4], in0=dotf[:, g4], scalar=-2.0,
                        in1=gf[:, g4], op0=Alu.mult, op1=Alu.add,
                    )
                    nc.vector.tensor_tensor(
                        out=dist2[:, g4], in0=s2a[:, g4], in1=hr_sb[:, g4],
                        op=Alu.add,
                    )
            if DEBUG:
                nc.sync.dma_start(dbg_bm[:], bm[:])
                nc.sync.dma_start(dbg_pf[:], pff[:])
                nc.sync.dma_start(dbg_dq[:], dq[:])
                nc.sync.dma_start(dbg_d2[:], dist2[:])
                nc.sync.dma_start(dbg_g[:, :, 0], gat[:, :, 128])
                nc.sync.dma_start(dbg_g[:, :, 1], gat[:, :, 129])
                nc.sync.dma_start(dbg_g[:, :, 2], gat[:, :, 0])
                nc.sync.dma_start(dbg_g[:, :, 3], gat[:, :, D])
            lns = small.tile([P, NT], f32)
            nc.scalar.activation(lns[:], dist2[:], Act.Ln)
            kol = small.tile([P, NT], f32)
            # koleo = relu(-0.5*ln(dist^2) - ln(N))
            nc.scalar.activation(
                kol[:], lns[:], Act.Relu, bias=mlnn_sb[:, 0:1], scale=-0.5
            )
            nc.sync.dma_start(out_d[:], kol[:])

    nc.compile()
    _built["nc"] = nc
    return nc


def _prep_in_maps(v: np.ndarray) -> list[dict]:
    bf = ml_dtypes.bfloat16
    n2_all = None
    in_maps = []
    jj = np.arange(ROWS)
    for c in range(NCORES):
        vr = np.roll(v, -c * ROWS, axis=0)
        w = (vr[0::2] + vr[1::2]) * np.float32(SCALE)  # [NPAIR, D]
        u = (vr[0::2] - vr[1::2]) * np.float32(SCALE)
        rows = vr[:ROWS]
        wt = np.ascontiguousarray(w.T).astype(bf)
        ut = np.ascontiguousarray(u.T).astype(bf)
        rt = np.ascontiguousarray(rows.T).astype(bf)
        rsb = rows.reshape(NT, P, D).transpose(1, 0, 2)

        n2 = np.sum(vr.astype(np.float64) * vr, axis=1).astype(np.float32)
        Sv = np.sum(vr.astype(np.float64), axis=1).astype(np.float32)
        gvec = n2 - np.float32(2.0 * EPS) * Sv  # per-row: ||x||^2 - 2 eps sum(x)

        vpair = np.zeros((NPAIR, VREC), dtype=np.float32)
        vpair[:, 0:D] = vr[0::2]
        vpair[:, D : 2 * D] = vr[1::2]
        vpair[:, 2 * D] = gvec[0::2]
        vpair[:, 2 * D + 1] = gvec[1::2]

        hr = (
            n2[:ROWS]
            + np.float32(2.0 * EPS) * Sv[:ROWS]
            + np.float32(D * EPS * EPS)
        )
        pidx = jj ^ 1
        pdot = np.sum(
            rows.astype(np.float64) * vr[pidx], axis=1
        ).astype(np.float32)
        pgv = gvec[pidx]

        in_maps.append(
            {
                "wt": np.concatenate([wt, wt], axis=0),
                "ut": np.concatenate([ut, ut], axis=0),
                "vrows_t": np.concatenate([rt, rt], axis=0),
                "vr2": np.ascontiguousarray(np.concatenate([rsb, rsb], axis=2)),
                "vpair": vpair,
                "hr": np.ascontiguousarray(hr.reshape(NT, P).T),
                "pd": np.ascontiguousarray(pdot.reshape(NT, P).T),
                "pg": np.ascontiguousarray(pgv.reshape(NT, P).T),
            }
        )
    return in_maps


# test.py can flip these to profile the run
TRACE = False
DEBUG = False
DEBUG_NO_CLAMP = False
LAST_RESULT = {}


def kernel(latents: np.ndarray) -> np.ndarray:
    from concourse.bass_utils import run_bass_kernel_spmd

    v = np.asarray(latents, dtype=np.float32).reshape(N, D)
    nc = _build_nc()
    in_maps = _prep_in_maps(v)

    kwargs = {}
    if TRACE:
        kwargs = dict(trace=True, stitch_traces=False)
    res = run_bass_kernel_spmd(nc, in_maps, core_ids=list(range(NCORES)), **kwargs)
    LAST_RESULT["res"] = res

    vals = np.concatenate([r["out"].reshape(-1) for r in res.results])
    return np.array(np.mean(vals), dtype=np.float32)


# revision 31
# speedup vs baseline: 1.2589x; 1.0498x over previous
"""KoLeo loss kernel for 8 Trainium2 NeuronCores.

Reference semantics:
    v = latents.squeeze()            # [N, D] f32, N=16384, D=64
    dp = v @ v.T ; dp[i,i] = -1      # NxN scores, diagonal excluded
    idx = argmax(dp, axis=1)         # nearest neighbor by dot product
    dist = ||v - v[idx] + 1e-6||_2
    out = mean(relu(-log(dist * N)))

Sharding: rows are block-sharded 2048/core.  Each core gets a copy of v
whose rows are ROTATED by -core*2048, so the self-match diagonal of its
local [2048, 16384] score block always lands at pair-column (row//2) --
the SPMD program is identical on all cores.

Pairwise-max trick: max(a, b) = (a + b + |a-b|) / 2.  The host ships
paired sums w = v[0::2]+v[1::2] and diffs u = v[0::2]-v[1::2], both
PRE-SCALED by 2^17; the PE computes dp-sums s = rows @ w.T and dp-diffs
d = rows @ u.T.  ScalarE takes |d| out of PSUM (its only bulk job), and
a fused custom VectorE op consumes (s from PSUM, |d| from SBUF) at one
output/cycle -- i.e. TWO dp elements per DVE cycle:

    pack = round_16384(s + |d|) + global_pair_idx;  accum = max

The fp32 magic-constant rounding makes the pack exact; the scan carries
the GLOBAL pair index (wave base via the scan init), and the accumulator
is chained across waves through its seed operand, so a row-tile's final
bm value directly encodes (quantized pair-max, winning pair index) over
all 8192 pairs -- no per-chunk argmax recovery pass is needed.

The self-pair is excluded by accumulating -2^30 into the SUM stream at
the self position (one extra matmul per row-tile, wave 0 only); the
partner (the excluded pair's other member) is reinstated as an extra
candidate in the tail from host-precomputed partner dot/norm values.

Tail: one indirect gather per two row-tiles fetches the winning pair's
two member vectors plus their precomputed (norm - 2*eps*sum) values;
exact dots pick the member, a host-precomputed partner candidate is
merged, and dist^2 = hr + g - 2*dot, then relu(-0.5*ln - ln N).
Host: mean of the 8x2048 per-row values.
"""

import math

import ml_dtypes
import numpy as np

N = 16384
D = 64
NCORES = 8
ROWS = N // NCORES  # 2048 rows per core
P = 128  # partitions
NT = ROWS // P  # 16 row-tiles per core
NPAIR = N // 2  # 8192 pair columns
WCH = 1024  # pair columns per scan wave (2 PSUM banks)
NW = NPAIR // WCH  # 8 waves per row-tile
VREC = 160  # gather record width (f32): 640B, 64B-aligned
MM_N = 512  # matmul free dim (1 PSUM bank)
EPS = 1.0e-6

SCALE_R = 4096.0  # 2^12 pre-scale baked into vrows on the host
SCALE_W = 32.0  # 2^5 pre-scale baked into fp8 wt/ut (product 2^17)
BIG = 2.0**30  # diagonal suppression on the (scaled) SUM stream
PACK_MAGIC = 3.0 * 2.0**22 * 16384.0  # rounds to multiples of 16384
UNPACK_MAGIC = 12582912.0  # 3 * 2^22: rounds to integers
NEGINF = -3.0e38

_OP_NAME = "KOLEO_PACK_GIDX"
_built = {}


def _register_pack_op():
    """Register the fused pair-max/global-argmax custom DVE op (idempotent).

    body  = round_16384(Src0 + Src1) + scan(+1, init=C1)
    accum = max, seeded from C0 (chains across waves via bm)
    Src0 = 2^17-scaled dp-sums (PSUM), Src1 = 2^17-scaled |dp-diffs| (SBUF).
    C1 = wave_base - 1 so the scan value is the GLOBAL pair index.
    C2 (imm2) = PACK_MAGIC.
    """
    from concourse import dve_ops
    from concourse.dve_spec import (
        AluOp, C0, C1, C2, One, Spec, Src0, Src1, lower, scan,
    )
    from concourse.dve_uop import DveOpSpec

    if _OP_NAME in dve_ops._SUB_OPCODE_FOR_NAME:
        return next(op for op in dve_ops.OPS if op.name == _OP_NAME)

    def _reference(in0, in1, s0, s1, imm2):
        p = in0.shape[0]
        s = in0.astype(np.float32).reshape(p, -1)
        a = in1.astype(np.float32).reshape(p, s.shape[1])
        z = (s + a) + np.float32(imm2) - np.float32(imm2)
        col = (np.float32(s1) + 1.0 + np.arange(s.shape[1], dtype=np.float32))[
            None, :
        ]
        body = (z + col).astype(np.float32)
        seed = np.asarray(s0, dtype=np.float32)
        seed = (
            seed.reshape(p, -1)[:, :1]
            if seed.ndim
            else np.full((p, 1), seed, dtype=np.float32)
        )
        acc = np.maximum(body.max(axis=-1, keepdims=True), seed)
        return body, acc

    body = ((Src0 + Src1) + C2 - C2) + scan(AluOp.ADD, One, init=C1)
    spec = Spec(
        body=body, accum=AluOp.MAX, accum_init=C0, reference=_reference
    )

    row = max(dve_ops._SUB_OPCODE_FOR_NAME.values()) + 1
    shas = {}
    for ver in ("v3", "v4"):
        uops = lower(spec, ver=ver)
        shas[ver] = DveOpSpec(
            name=_OP_NAME, opcode=row, uops=uops, rd1_en=True
        ).sha(ver)

    op = dve_ops.DveOp(_OP_NAME, spec, subdim=False, uops_sha=shas)
    dve_ops.OPS.append(op)
    dve_ops._SUB_OPCODE_FOR_NAME[_OP_NAME] = row
    dve_ops.CUSTOM_DVE_SPECS[_OP_NAME] = spec
    return op


def _build_nc():
    """Build + compile the per-core Bass program (same NEFF on all cores)."""
    if "nc" in _built:
        return _built["nc"]

    import concourse.bass as bass
    import concourse.mybir as mybir
    import concourse.tile as tile
    from concourse import bacc

    pack_op = _register_pack_op()

    f32 = mybir.dt.float32
    bf16 = mybir.dt.bfloat16
    i32 = mybir.dt.int32
    Alu = mybir.AluOpType
    Act = mybir.ActivationFunctionType

    nc = bacc.Bacc(None, target_bir_lowering=False)

    f8 = mybir.dt.float8e4
    # wt/ut/vrows_t are duplicated into partitions 64-127 for row-packing
    wt_d = nc.declare_dram_parameter("wt", [P, NPAIR], f8, isOutput=False)
    ut_d = nc.declare_dram_parameter("ut", [P, NPAIR], f8, isOutput=False)
    vrows_t = nc.declare_dram_parameter("vrows_t", [P, ROWS], bf16, isOutput=False)
    # rows duplicated along D so one op handles both gathered pair members
    vr2_d = nc.declare_dram_parameter("vr2", [P, NT, 2 * D], f32, isOutput=False)
    # pair record: [v[2p] | v[2p+1] | g_a | g_b | pad pad] (132 f32 = 528B)
    vpair_d = nc.declare_dram_parameter("vpair", [NPAIR, VREC], f32, isOutput=False)
    hr_d = nc.declare_dram_parameter("hr", [P, NT], f32, isOutput=False)
    pd_d = nc.declare_dram_parameter("pd", [P, NT], f32, isOutput=False)
    pg_d = nc.declare_dram_parameter("pg", [P, NT], f32, isOutput=False)
    out_d = nc.declare_dram_parameter("out", [P, NT], f32, isOutput=True)
    if DEBUG:
        dbg_bm = nc.declare_dram_parameter("dbg_bm", [P, NT], f32, isOutput=True)
        dbg_pf = nc.declare_dram_parameter("dbg_pf", [P, NT], f32, isOutput=True)
        dbg_dq = nc.declare_dram_parameter("dbg_dq", [P, NT, 2], f32, isOutput=True)
        dbg_d2 = nc.declare_dram_parameter("dbg_d2", [P, NT], f32, isOutput=True)
        dbg_g = nc.declare_dram_parameter("dbg_g", [P, NT, 4], f32, isOutput=True)

    neg_eye = nc.inline_tensor(
        (np.eye(P, dtype=np.float32) * -BIG).astype(ml_dtypes.bfloat16), "neg_eye"
    )
    half_np = np.zeros((P, P // 2), dtype=np.float32)
    half_np[np.arange(P), np.arange(P) // 2] = 1.0
    half_eye = nc.inline_tensor(half_np.astype(ml_dtypes.bfloat16), "half_eye")
    neginf_np = np.full((P, NT), NEGINF, dtype=np.float32)
    neginf_c = nc.inline_tensor(neginf_np, "neginf")
    mlnn_np = np.full((P, 1), -math.log(float(N)), dtype=np.float32)
    mlnn_c = nc.inline_tensor(mlnn_np, "mlnn")

    with tile.TileContext(nc) as tc:
        with (
            tc.tile_pool(name="consts", bufs=1) as consts,
            tc.tile_pool(name="psum", bufs=1, space="PSUM") as psum_pool,
            tc.tile_pool(name="work", bufs=1) as work,
            tc.tile_pool(name="small", bufs=1) as small,
        ):
            # ---- startup DMAs: strict consumption order, spread over all
            # five engine queues (PE/DVE queues are idle at kernel start)
            negI_sb = consts.tile([P, P], bf16)
            nc.scalar.dma_start(negI_sb[:], neg_eye[:])
            halfI_sb = consts.tile([P, P // 2], bf16)
            nc.scalar.dma_start(halfI_sb[:], half_eye[:])
            neginf_sb = consts.tile([P, NT], f32)
            nc.scalar.dma_start(neginf_sb[:], neginf_c[:])
            mlnn_sb = consts.tile([P, 1], f32)
            nc.scalar.dma_start(mlnn_sb[:], mlnn_c[:])
            vrt_sb = consts.tile([P, ROWS], bf16)
            nc.sync.dma_start(vrt_sb[:, 0:256], vrows_t[:, 0:256])
            wt_sb = consts.tile([P, NPAIR], f8)
            ut_sb = consts.tile([P, NPAIR], f8)
            w0 = slice(0, WCH)
            w1 = slice(WCH, 2 * WCH)
            nc.gpsimd.dma_start(wt_sb[:, w0], wt_d[:, w0])
            nc.scalar.dma_start(ut_sb[:, w0], ut_d[:, w0])
            nc.sync.dma_start(vrt_sb[:, 256:ROWS], vrows_t[:, 256:ROWS])
            nc.gpsimd.dma_start(wt_sb[:, w1], wt_d[:, w1])
            nc.scalar.dma_start(ut_sb[:, w1], ut_d[:, w1])
            engs = [nc.sync, nc.gpsimd, nc.scalar]
            q = 0
            for w in range(2, NW):
                sl = slice(w * WCH, (w + 1) * WCH)
                engs[q % 3].dma_start(wt_sb[:, sl], wt_d[:, sl])
                q += 1
                engs[q % 3].dma_start(ut_sb[:, sl], ut_d[:, sl])
                q += 1
            vr2_sb = consts.tile([P, NT, 2 * D], f32)
            nc.gpsimd.dma_start(vr2_sb[:], vr2_d[:])
            hr_sb = consts.tile([P, NT], f32)
            nc.sync.dma_start(hr_sb[:], hr_d[:])
            pd_sb = consts.tile([P, NT], f32)
            nc.scalar.dma_start(pd_sb[:], pd_d[:])
            pg_sb = consts.tile([P, NT], f32)
            nc.gpsimd.dma_start(pg_sb[:], pg_d[:])

            bm = small.tile([P, NT], f32)  # packed per-tile running maxima
            u1 = small.tile([P, NT], f32)
            u2 = small.tile([P, NT], f32)
            pff = small.tile([P, NT], f32)  # winning pair index (float)
            pfi = small.tile([P, NT], i32)  # winning pair index (int)
            gat = small.tile([P, NT, VREC], f32)  # gathered pair records
            prj = small.tile([P, NT, 2, D], f32)  # member products
            dq = small.tile([P, NT, 2], f32)  # member dots
            sel = small.tile([P, NT], f32)  # 1.0 if member b has larger dot
            dotw = small.tile([P, NT], f32)
            gdf = small.tile([P, NT], f32)
            gm = small.tile([P, NT], f32)
            pgd = small.tile([P, NT], f32)
            pgm = small.tile([P, NT], f32)
            gw = small.tile([P, NT], f32)
            c2 = small.tile([P, NT], f32)
            dotf = small.tile([P, NT], f32)
            gf = small.tile([P, NT], f32)
            s2a = small.tile([P, NT], f32)
            dist2 = small.tile([P, NT], f32)

            def wave_pair(s, w):
                tA, tB = 2 * s, 2 * s + 1
                lhsA = vrt_sb[0:64, tA * P : (tA + 1) * P]
                lhsB = vrt_sb[64:128, tB * P : (tB + 1) * P]
                if True:
                    # A uses PE rows 0-63, B rows 64-127: adjacent MMs
                    # alternate quadrants so pairs run concurrently.  The 4
                    # single-buffered tag-tiles fill all 8 PSUM banks; A and
                    # B act as each other's double buffer in the pipeline.
                    psSA = psum_pool.tile([P, WCH], f32, tag="psSA", bufs=1)
                    psSB = psum_pool.tile([P, WCH], f32, tag="psSB", bufs=1)
                    psDA = psum_pool.tile([P, WCH], f32, tag="psDA", bufs=1)
                    psDB = psum_pool.tile([P, WCH], f32, tag="psDB", bufs=1)
                    for h in range(WCH // MM_N):
                        csl = slice(w * WCH + h * MM_N, w * WCH + (h + 1) * MM_N)
                        osl = slice(h * MM_N, (h + 1) * MM_N)
                        nc.tensor.matmul(
                            psDA[:, osl], lhsA, ut_sb[0:64, csl],
                            start=True, stop=True,
                        )
                        nc.tensor.matmul(
                            psDB[:, osl], lhsB, ut_sb[64:128, csl],
                            start=True, stop=True,
                        )
                    absA = work.tile([P, WCH], f32, tag="absD", bufs=4)
                    nc.scalar.activation(absA[:], psDA[:], Act.Abs)
                    for h in range(WCH // MM_N):
                        csl = slice(w * WCH + h * MM_N, w * WCH + (h + 1) * MM_N)
                        osl = slice(h * MM_N, (h + 1) * MM_N)
                        # the self-pair block (cols [t*64, t*64+64)) is
                        # always inside wave 0; keep that MM's group open
                        dhA = w == 0 and h == (tA * 64) // MM_N
                        dhB = w == 0 and h == (tB * 64) // MM_N
                        nc.tensor.matmul(
                            psSA[:, osl], lhsA, wt_sb[0:64, csl],
                            start=True, stop=not dhA,
                        )
                        nc.tensor.matmul(
                            psSB[:, osl], lhsB, wt_sb[64:128, csl],
                            start=True, stop=not dhB,
                        )
                    if w == 0:
                        offA, offB = tA * 64, tB * 64
                        nc.tensor.matmul(
                            psSA[:, offA : offA + 64], negI_sb[:], halfI_sb[:],
                            start=False, stop=True,
                        )
                        nc.tensor.matmul(
                            psSB[:, offB : offB + 64], negI_sb[:], halfI_sb[:],
                            start=False, stop=True,
                        )
                    absB = work.tile([P, WCH], f32, tag="absD", bufs=4)
                    nc.scalar.activation(absB[:], psDB[:], Act.Abs)
                    for t, psS, absD in ((tA, psSA, absA), (tB, psSB, absB)):
                        junk = work.tile([P, WCH], f32, tag="junk", bufs=2)
                        seed = (
                            neginf_sb[:, t : t + 1]
                            if w == 0
                            else bm[:, t : t + 1]
                        )
                        nc.vector._custom_dve(
                            pack_op, out=junk[:], in0=psS[:], in1=absD[:],
                            s0=seed, s1=float(w * WCH - 1), imm2=PACK_MAGIC,
                            accum_out=bm[:, t : t + 1],
                        )
                        if w == NW - 1:
                            # tile finished: recover its winning pair index and
                            # launch the gather before the next tile's scan
                            ts1 = slice(t, t + 1)
                            nc.scalar.activation(
                                u1[:, ts1], bm[:, ts1], Act.Copy,
                                bias=UNPACK_MAGIC, scale=1.0 / 16384.0,
                            )
                            nc.scalar.activation(
                                u2[:, ts1], u1[:, ts1], Act.Copy,
                                bias=16384.0 * UNPACK_MAGIC, scale=-16384.0,
                            )
                            nc.vector.tensor_tensor(
                                out=pff[:, ts1], in0=bm[:, ts1],
                                in1=u2[:, ts1], op=Alu.add,
                            )
                            nc.vector.tensor_copy(pfi[:, ts1], pff[:, ts1])
                            nc.gpsimd.indirect_dma_start(
                                out=gat[:, t, :], out_offset=None, in_=vpair_d[:],
                                in_offset=bass.IndirectOffsetOnAxis(
                                    ap=pfi[:, ts1], axis=0
                                ),
                            )

            for s in range(NT // 2):
                tA, tB = 2 * s, 2 * s + 1
                for w in range(NW):
                    wave_pair(s, w)

                if s % 2 == 1:
                    # ---- exact dots + candidate combine for the last 4 tiles
                    g4 = slice(tA - 2, tB + 1)
                    nc.vector.tensor_tensor(
                        out=prj[:, g4, :, :],
                        in0=vr2_sb[:, g4, :],
                        in1=gat[:, g4, 0 : 2 * D],
                        op=Alu.mult,
                    )
                    nc.vector.tensor_reduce(
                        dq[:, g4, :], prj[:, g4, :, :],
                        axis=mybir.AxisListType.X, op=Alu.add,
                    )
                    # member with larger dot; its g via predicated copy
                    nc.vector.tensor_tensor(
                        out=sel[:, g4], in0=dq[:, g4, 1], in1=dq[:, g4, 0],
                        op=Alu.is_gt,
                    )
                    nc.vector.tensor_tensor(
                        out=dotw[:, g4], in0=dq[:, g4, 0], in1=dq[:, g4, 1],
                        op=Alu.max,
                    )
                    nc.vector.tensor_tensor(
                        out=gdf[:, g4], in0=gat[:, g4, 2 * D + 1],
                        in1=gat[:, g4, 2 * D], op=Alu.subtract,
                    )
                    nc.vector.tensor_tensor(
                        out=gm[:, g4], in0=gdf[:, g4], in1=sel[:, g4], op=Alu.mult
                    )
                    nc.vector.tensor_tensor(
                        out=gw[:, g4], in0=gat[:, g4, 2 * D], in1=gm[:, g4],
                        op=Alu.add,
                    )
                    # partner candidate wins if its (host-exact) dot is larger
                    nc.vector.tensor_tensor(
                        out=c2[:, g4], in0=pd_sb[:, g4], in1=dotw[:, g4],
                        op=Alu.is_gt,
                    )
                    nc.vector.tensor_tensor(
                        out=dotf[:, g4], in0=dotw[:, g4], in1=pd_sb[:, g4],
                        op=Alu.max,
                    )
                    nc.vector.tensor_tensor(
                        out=pgd[:, g4], in0=pg_sb[:, g4], in1=gw[:, g4],
                        op=Alu.subtract,
                    )
                    nc.vector.tensor_tensor(
                        out=pgm[:, g4], in0=pgd[:, g4], in1=c2[:, g4], op=Alu.mult
                    )
                    nc.vector.tensor_tensor(
                        out=gf[:, g4], in0=gw[:, g4], in1=pgm[:, g4], op=Alu.add
                    )
                    # dist^2 = hr + g - 2*dot
                    nc.vector.scalar_tensor_tensor(
                        out=s2a[:, g4], in0=dotf[:, g4], scalar=-2.0,
                        in1=gf[:, g4], op0=Alu.mult, op1=Alu.add,
                    )
                    nc.vector.tensor_tensor(
                        out=dist2[:, g4], in0=s2a[:, g4], in1=hr_sb[:, g4],
                        op=Alu.add,
                    )
            if DEBUG:
                nc.sync.dma_start(dbg_bm[:], bm[:])
                nc.sync.dma_start(dbg_pf[:], pff[:])
                nc.sync.dma_start(dbg_dq[:], dq[:])
                nc.sync.dma_start(dbg_d2[:], dist2[:])
                nc.sync.dma_start(dbg_g[:, :, 0], gat[:, :, 128])
                nc.sync.dma_start(dbg_g[:, :, 1], gat[:, :, 129])
                nc.sync.dma_start(dbg_g[:, :, 2], gat[:, :, 0])
                nc.sync.dma_start(dbg_g[:, :, 3], gat[:, :, D])
            lns = small.tile([P, NT], f32)
            nc.scalar.activation(lns[:], dist2[:], Act.Ln)
            kol = small.tile([P, NT], f32)
            # koleo = relu(-0.5*ln(dist^2) - ln(N))
            nc.scalar.activation(
                kol[:], lns[:], Act.Relu, bias=mlnn_sb[:, 0:1], scale=-0.5
            )
            nc.sync.dma_start(out_d[:], kol[:])

    nc.compile()
    _built["nc"] = nc
    return nc


def _prep_in_maps(v: np.ndarray) -> list[dict]:
    bf = ml_dtypes.bfloat16
    f8 = ml_dtypes.float8_e4m3
    in_maps = []
    jj = np.arange(ROWS)
    for c in range(NCORES):
        vr = np.roll(v, -c * ROWS, axis=0)
        w = (vr[0::2] + vr[1::2]) * np.float32(SCALE_W)  # [NPAIR, D]
        u = (vr[0::2] - vr[1::2]) * np.float32(SCALE_W)
        rows = vr[:ROWS]
        wt = np.ascontiguousarray(w.T).astype(f8)
        ut = np.ascontiguousarray(u.T).astype(f8)
        rt = np.ascontiguousarray(rows.T * np.float32(SCALE_R)).astype(bf)
        rsb = rows.reshape(NT, P, D).transpose(1, 0, 2)

        n2 = np.sum(vr.astype(np.float64) * vr, axis=1).astype(np.float32)
        Sv = np.sum(vr.astype(np.float64), axis=1).astype(np.float32)
        gvec = n2 - np.float32(2.0 * EPS) * Sv  # per-row: ||x||^2 - 2 eps sum(x)

        vpair = np.zeros((NPAIR, VREC), dtype=np.float32)
        vpair[:, 0:D] = vr[0::2]
        vpair[:, D : 2 * D] = vr[1::2]
        vpair[:, 2 * D] = gvec[0::2]
        vpair[:, 2 * D + 1] = gvec[1::2]

        hr = (
            n2[:ROWS]
            + np.float32(2.0 * EPS) * Sv[:ROWS]
            + np.float32(D * EPS * EPS)
        )
        pidx = jj ^ 1
        pdot = np.sum(
            rows.astype(np.float64) * vr[pidx], axis=1
        ).astype(np.float32)
        pgv = gvec[pidx]

        in_maps.append(
            {
                "wt": np.concatenate([wt, wt], axis=0),
                "ut": np.concatenate([ut, ut], axis=0),
                "vrows_t": np.concatenate([rt, rt], axis=0),
                "vr2": np.ascontiguousarray(np.concatenate([rsb, rsb], axis=2)),
                "vpair": vpair,
                "hr": np.ascontiguousarray(hr.reshape(NT, P).T),
                "pd": np.ascontiguousarray(pdot.reshape(NT, P).T),
                "pg": np.ascontiguousarray(pgv.reshape(NT, P).T),
            }
        )
    return in_maps


# test.py can flip these to profile the run
TRACE = False
DEBUG = False
DEBUG_NO_CLAMP = False
LAST_RESULT = {}


def kernel(latents: np.ndarray) -> np.ndarray:
    from concourse.bass_utils import run_bass_kernel_spmd

    v = np.asarray(latents, dtype=np.float32).reshape(N, D)
    nc = _build_nc()
    in_maps = _prep_in_maps(v)

    kwargs = {}
    if TRACE:
        kwargs = dict(trace=True, stitch_traces=False)
    res = run_bass_kernel_spmd(nc, in_maps, core_ids=list(range(NCORES)), **kwargs)
    LAST_RESULT["res"] = res

    vals = np.concatenate([r["out"].reshape(-1) for r in res.results])
    return np.array(np.mean(vals), dtype=np.float32)
